# revision 7
# baseline (speedup 1.0000x reference)
"""Trainium2 Bass kernel for nn_DSA2Attention (MLA-latent sparse sliding-window attention).

Strategy (tensor-parallel over heads, 8 cores, 2 heads/core):
  host:  fold Wc into Wk/Wv (k = x @ (Wc@Wk) etc), permute q/k head-dims so rope
         pairs become [x1(0:64); x2(64:128)], precompute rope cos/sin tables in
         [d', t] layout, transposed triangle masks (bf16), a block-selector
         matrix sel_rep, identity.
  device (per core, SPMD — identical program, different weight slices):
    phase 1: qT[d,t], kT[d,t] (feature-major) and v[t,d] via PE matmuls from
             xT chunks (DMA interleaved per k-tile so the PE starts early);
             rope on DVE; block-mean kbT via segmented reduce.
    bulk:    per (qt, h): block scores bsc = qT.T@kbT; top-16-of-32 via
             max8/match_replace; boost vector -> boost_all; every 4 chunks
             PE-transpose boost_all -> boostT (for the boost matmul below).
    phase 2: per query-tile of 128: S = qT.T@kT over <=5 key tiles in PSUM;
             boost broadcast-add and triangle masks are MATMULS accumulated
             into the same PSUM group (lhsT=boostT slice, rhs=sel_rep slice;
             lhsT=triT, rhs=I) — no DVE/GpSimd touch of S;
             exp (scores bounded; no row-max) PSUM->SBUF bf16 with accumulated
             row sums; normalize on DVE (bf16 4x mode); P.T via xbar DMA
             transpose (SBUF->SBUF, no PE); AV accumulate attnT[d,q];
             out-projection psum -> DMA straight to DRAM.
  host:  sum the 8 partial projections (row-parallel Wo) + bias.

Numerics: matmul operands in bf16 (fp32 PSUM accumulation), softmax chain in
fp32 except P in bf16, output partials bf16 summed in fp64 on host.
"""
import os
import numpy as np

import concourse.bacc as bacc
import concourse.bass as bass
import concourse.mybir as mybir
import concourse.tile as tile
from concourse.bass_utils import run_bass_kernel_spmd

B, T, D = 1, 2048, 2048
NH, NKV, HD = 16, 4, 128
KVC = 512
WIN = 512
BS = 64
NSEL = 16
SCALE = HD ** -0.5
NB = T // BS          # 32
NCORE = 8
HPC = NH // NCORE     # heads per core = 2

KT = T // 128         # 16 k-tiles
NCH = 4               # phase-1 t-chunks
CH = T // NCH         # 512
QT = T // 128         # 16 query tiles
NEG = -1e30
MASKV = -1e30 / SCALE

F32 = mybir.dt.float32
AF = mybir.ActivationFunctionType
OP = mybir.AluOpType

MM_DT = os.environ.get("MM_DT", "bf16")
MMDT = {"bf16": mybir.dt.bfloat16, "f32": F32}[MM_DT]

_cache = {}


def build_nc(trace_label=""):
    nc = bacc.Bacc("TRN2", target_bir_lowering=False, debug=False, num_devices=NCORE)

    xT_d = nc.dram_tensor("xT", [D, T], MMDT, kind="ExternalInput").ap()
    wq_d = nc.dram_tensor("wq", [D, HPC * HD], MMDT, kind="ExternalInput").ap()
    wck_d = nc.dram_tensor("wck", [D, HD], MMDT, kind="ExternalInput").ap()
    wcv_d = nc.dram_tensor("wcv", [D, HD], MMDT, kind="ExternalInput").ap()
    wo_d = nc.dram_tensor("wo", [HPC * HD, D], MMDT, kind="ExternalInput").ap()
    cos2_d = nc.dram_tensor("cos2", [HD, T], F32, kind="ExternalInput").ap()
    sin2_d = nc.dram_tensor("sin2", [HD, T], F32, kind="ExternalInput").ap()
    trihiT_d = nc.dram_tensor("trihiT", [128, 128], MMDT, kind="ExternalInput").ap()
    triloT_d = nc.dram_tensor("triloT", [128, 128], MMDT, kind="ExternalInput").ap()
    selrep_d = nc.dram_tensor("selrep", [128, T], MMDT, kind="ExternalInput").ap()
    eye_d = nc.dram_tensor("eye", [128, 128], F32, kind="ExternalInput").ap()
    eyeb_d = nc.dram_tensor("eyeb", [128, 128], MMDT, kind="ExternalInput").ap()
    jt_d = nc.dram_tensor("jt", [128, 128], MMDT, kind="ExternalInput").ap()
    bias3_d = nc.dram_tensor("bias3", [HD, 3], F32, kind="ExternalInput").ap()
    out_d = nc.dram_tensor("out", [T, D], mybir.dt.bfloat16,
                           kind="ExternalOutput").ap()

    with tile.TileContext(nc) as tc:
        with tc.tile_pool(name="persist", bufs=1) as pp:
            wq_sb = pp.tile([128, KT * HPC * HD], MMDT, tag="wq")
            wck_sb = pp.tile([128, KT * HD], MMDT, tag="wck")
            wcv_sb = pp.tile([128, KT * HD], MMDT, tag="wcv")
            wo_sb = pp.tile([128, HPC * D], MMDT, tag="wo")
            cos2_sb = pp.tile([128, T], F32, tag="cos2")
            sin2_sb = pp.tile([128, T], F32, tag="sin2")
            trihiT = pp.tile([128, 128], MMDT, tag="trihiT")
            triloT = pp.tile([128, 128], MMDT, tag="triloT")
            selrep = pp.tile([128, T], MMDT, tag="selrep")
            eye_sb = pp.tile([128, 128], F32, tag="eye")
            eyeb = pp.tile([128, 128], MMDT, tag="eyeb")
            jt_sb = pp.tile([128, 128], MMDT, tag="jt")
            bias3 = pp.tile([128, 3], F32, tag="bias3")
            qT_sb = pp.tile([128, HPC * T], MMDT, tag="qT")
            kT_sb = pp.tile([128, T], MMDT, tag="kT")
            v_sb = pp.tile([128, KT * HD], MMDT, tag="v")
            kbT = pp.tile([128, NB], MMDT, tag="kbT")
            boost_all = pp.tile([128, QT * HPC * NB], F32, tag="boost_all")
            boostT = pp.tile([128, QT * HPC * NB], MMDT, tag="boostT")

            # ---------- phase 1 DMA: interleave weights and x per k-tile so
            # the first matmuls can start after ~200KB instead of ~3MB.
            xt_sb = pp.tile([128, KT * T], MMDT, tag="xt")
            vT_sb = pp.tile([128, T], MMDT, tag="vT")
            for kt in range(KT):
                nc.sync.dma_start(
                    wq_sb[:, kt * HPC * HD:(kt + 1) * HPC * HD],
                    wq_d[kt * 128:(kt + 1) * 128, :])
                nc.sync.dma_start(
                    wck_sb[:, kt * HD:(kt + 1) * HD],
                    wck_d[kt * 128:(kt + 1) * 128, :])
                nc.sync.dma_start(
                    wcv_sb[:, kt * HD:(kt + 1) * HD],
                    wcv_d[kt * 128:(kt + 1) * 128, :])
                nc.sync.dma_start(
                    xt_sb[:, kt * T:kt * T + CH],
                    xT_d[kt * 128:(kt + 1) * 128, 0:CH])
            nc.sync.dma_start(bias3[:], bias3_d)
            nc.sync.dma_start(cos2_sb[:], cos2_d)
            nc.sync.dma_start(sin2_sb[:], sin2_d)
            nc.sync.dma_start(jt_sb[:], jt_d)
            nc.sync.dma_start(trihiT[:], trihiT_d)
            nc.sync.dma_start(triloT[:], triloT_d)
            nc.sync.dma_start(selrep[:], selrep_d)
            nc.sync.dma_start(eye_sb[:], eye_d)
            nc.sync.dma_start(eyeb[:], eyeb_d)
            for ch in range(1, NCH):
                for kt in range(KT):
                    nc.sync.dma_start(
                        xt_sb[:, kt * T + ch * CH:kt * T + (ch + 1) * CH],
                        xT_d[kt * 128:(kt + 1) * 128, ch * CH:(ch + 1) * CH])
            nc.sync.dma_start(
                wo_sb[:].rearrange("p (h e) -> p h e", h=HPC),
                wo_d.rearrange("(h p) e -> p h e", p=128))

            # ---------- phase 1 compute: qT, kT, v ----------
            with tc.tile_pool(name="rs", bufs=3) as rsp, \
                 tc.tile_pool(name="psA", bufs=8, space="PSUM") as psA:
                def p1_mms(ch):
                    qd = [psA.tile([128, CH], F32, tag="qkT", name=f"qd{ch}_{_h}")
                          for _h in range(HPC)]
                    kTp = psA.tile([128, CH], F32, tag="qkT", name=f"kTp{ch}")
                    vTp = psA.tile([128, CH], F32, tag="qkT", name=f"vTp{ch}")
                    for kt in range(KT):
                        xt = xt_sb[:, kt * T + ch * CH:kt * T + (ch + 1) * CH]
                        st = dict(start=(kt == 0), stop=(kt == KT - 1))
                        for h in range(HPC):
                            nc.tensor.matmul(
                                qd[h][:],
                                lhsT=wq_sb[:, kt * HPC * HD + h * HD:
                                           kt * HPC * HD + (h + 1) * HD],
                                rhs=xt, **st)
                        nc.tensor.matmul(
                            kTp[:], lhsT=wck_sb[:, kt * HD:(kt + 1) * HD],
                            rhs=xt, **st)
                        nc.tensor.matmul(
                            vTp[:], lhsT=wcv_sb[:, kt * HD:(kt + 1) * HD],
                            rhs=xt, **st)
                    return qd, kTp, vTp

                def p1_rope(ch, qd, kTp, vTp):
                    cs = slice(ch * CH, (ch + 1) * CH)
                    # rope + bias: dst = (ps+b)*cos2 + J @ ((ps+b)*sin2)
                    for ti, (ps, dst) in enumerate(
                            [(qd[0], qT_sb[:, 0 * T + ch * CH:0 * T + (ch + 1) * CH]),
                             (qd[1], qT_sb[:, 1 * T + ch * CH:1 * T + (ch + 1) * CH]),
                             (kTp, kT_sb[:, cs])]):
                        U = rsp.tile([128, CH], F32, tag="U", name=f"U{ch}_{ti}")
                        Wt = rsp.tile([128, CH], MMDT, tag="W", name=f"Wt{ch}_{ti}")
                        b = bias3[:, ti:ti + 1]
                        nc.vector.scalar_tensor_tensor(
                            U[:], ps[:], b, cos2_sb[:, cs], op0=OP.add, op1=OP.mult)
                        nc.vector.scalar_tensor_tensor(
                            Wt[:], ps[:], b, sin2_sb[:, cs], op0=OP.add, op1=OP.mult)
                        rp = psA.tile([128, CH], F32, tag="qkT", name=f"rp{ch}_{ti}")
                        nc.tensor.matmul(rp[:], lhsT=jt_sb[:], rhs=Wt[:],
                                         start=True, stop=True)
                        nc.vector.tensor_add(dst, rp[:], U[:])
                    nc.any.tensor_copy(vT_sb[:, cs], vTp[:])

                prev = None
                for ch in range(NCH):
                    cur = p1_mms(ch)
                    if prev is not None:
                        p1_rope(ch - 1, *prev)
                    prev = cur
                p1_rope(NCH - 1, *prev)

                # v[t, d] from vT[d, t] via one xbar transpose (bf16)
                nc.sync.dma_start_transpose(
                    v_sb[:].rearrange("p (k f) -> p k f", k=KT), vT_sb[:])

                # block means of roped kT: [128, T] -> [128, NB], 1/BS scale
                with nc.allow_low_precision(reason="bf16 block-mean output"):
                    nc.vector.reduce_sum(
                        kbT[:, :, None],
                        kT_sb[:].rearrange("p (b i) -> p b i", b=NB),
                        axis=mybir.AxisListType.X)
                nc.vector.tensor_scalar_mul(kbT[:], kbT[:], 1.0 / BS)

            # ---------- bulk: block scores + top-16 boost + boostT ----------
            # qt-major chunk order: chunk c = qt*HPC + h, so PE transpose t
            # (cols [128t,128t+128) = chunks 4t..4t+3) is ready for qt pair.
            with tc.tile_pool(name="psB", bufs=4, space="PSUM") as psB, \
                 tc.tile_pool(name="psT", bufs=2, space="PSUM") as psT, \
                 tc.tile_pool(name="pTk", bufs=6) as pTk:
                for qt in range(QT):
                    for h in range(HPC):
                        c = qt * HPC + h
                        qTh = qT_sb[:, h * T + qt * 128:h * T + (qt + 1) * 128]
                        bsc = psB.tile([128, NB], F32, tag="bsc",
                                       name=f"bsc{qt}_{h}")
                        nc.tensor.matmul(bsc[:], lhsT=qTh, rhs=kbT[:],
                                         start=True, stop=True)
                        z = pTk.tile([128, NB], F32, tag="z", name=f"z{qt}_{h}")
                        m8 = pTk.tile([128, 8], F32, tag="m8",
                                      name=f"m8_{qt}_{h}")
                        nc.vector.tensor_copy(z[:], bsc[:])
                        for _ in range(NSEL // 8):
                            nc.vector.max(out=m8[:], in_=z[:])
                            nc.vector.match_replace(
                                out=z[:], in_to_replace=m8[:], in_values=z[:],
                                imm_value=NEG)
                        bo_sl = boost_all[:, c * NB:(c + 1) * NB]
                        nc.vector.scalar_tensor_tensor(
                            bo_sl, z[:], NEG, bsc[:],
                            op0=OP.is_le, op1=OP.mult)
                    if qt % 2 == 1:
                        t = qt // 2
                        btp = psT.tile([128, 128], F32, tag="btp", name=f"btp{t}")
                        nc.tensor.transpose(
                            btp[:], boost_all[:, t * 128:(t + 1) * 128], eye_sb[:])
                        nc.vector.tensor_copy(
                            boostT[:, t * 128:(t + 1) * 128], btp[:])

            # ---------- phase 2: attention + projection ----------
            # Stage A(qt): S qk matmuls + boost/mask matmuls in one PSUM
            # group -> exp (PSUM->SBUF bf16, accum row sums) -> DVE normalize
            # (bf16 4x) -> xbar DMA transpose P -> PnT.
            # Stage B(qt): AV -> projection -> DMA out.
            with tc.tile_pool(name="psS", bufs=3, space="PSUM") as psS, \
                 tc.tile_pool(name="psAcc", bufs=2, space="PSUM") as psAcc, \
                 tc.tile_pool(name="pPn", bufs=3) as pPn, \
                 tc.tile_pool(name="pPT", bufs=2) as pPT, \
                 tc.tile_pool(name="pA", bufs=4) as pA, \
                 tc.tile_pool(name="pOut", bufs=2) as pOut, \
                 tc.tile_pool(name="pSm", bufs=8) as pSm:
                pnt_tiles = {}

                def stage_a(qt):
                    nk = min(qt, 4) + 1
                    kt0 = qt + 1 - nk
                    t = qt // 2
                    olist = [(0, 512), (512, 128)] if nk == 5 else [(0, nk * 128)]
                    S = [psS.tile([128, 640], F32, tag="S", name=f"S{qt}_{h}")
                         for h in range(HPC)]
                    # qk matmuls (one LDW per head)
                    for h in range(HPC):
                        qTh = qT_sb[:, h * T + qt * 128:h * T + (qt + 1) * 128]
                        for o, w in olist:
                            nc.tensor.matmul(
                                S[h][:, o:o + w], lhsT=qTh,
                                rhs=kT_sb[:, kt0 * 128 + o:kt0 * 128 + o + w],
                                start=True, stop=False)
                    # triangle masks as accumulated matmuls (lhsT=triT, rhs=I)
                    if qt >= 4:
                        for h in range(HPC):
                            nc.tensor.matmul(
                                S[h][:, 0:128], lhsT=trihiT[:], rhs=eyeb[:],
                                start=False, stop=False)
                    for h in range(HPC):
                        nc.tensor.matmul(
                            S[h][:, (nk - 1) * 128:nk * 128], lhsT=triloT[:],
                            rhs=eyeb[:], start=False, stop=False)
                    # boost broadcast-add as matmul: lhsT = boostT chunk
                    # [32, 128q], rhs = selector rows at same partitions
                    for h in range(HPC):
                        c = qt * HPC + h
                        s = (c % 4) * 32
                        bT = boostT[s:s + 32, t * 128:(t + 1) * 128]
                        for o, w in olist:
                            nc.tensor.matmul(
                                S[h][:, o:o + w], lhsT=bT,
                                rhs=selrep[s:s + 32,
                                           kt0 * 128 + o:kt0 * 128 + o + w],
                                start=False, stop=True,
                                tile_position=(s, 0))
                    # exp + row sums; normalize on DVE (bf16 4x); transpose
                    Pn = pPn.tile([128, 2 * 640], MMDT, tag="Pn", name=f"Pn{qt}")
                    PnT = pPT.tile([128, 2 * 640], MMDT, tag="PnT", name=f"PnT{qt}")
                    pnt_tiles[qt] = PnT
                    for h in range(HPC):
                        r = pSm.tile([128, 1], F32, tag="r", name=f"r{qt}_{h}")
                        rinv = pSm.tile([128, 1], F32, tag="rinv",
                                        name=f"rinv{qt}_{h}")
                        pn_h = Pn[:, h * 640:h * 640 + nk * 128]
                        with nc.allow_low_precision(reason="bf16 P with f32 sums"):
                            nc.scalar.activation(
                                pn_h, S[h][:, :nk * 128], AF.Exp,
                                scale=float(SCALE), accum_out=r[:])
                        nc.vector.reciprocal(rinv[:], r[:])
                        nc.vector.tensor_scalar_mul(pn_h, pn_h, rinv[:])
                        nc.sync.dma_start_transpose(
                            PnT[:, h * 640:h * 640 + nk * 128].rearrange(
                                "p (k f) -> p k f", k=nk),
                            pn_h)

                def stage_b(qt):
                    nk = min(qt, 4) + 1
                    kt0 = qt + 1 - nk
                    PnT = pnt_tiles.pop(qt)
                    attnT = [psAcc.tile([128, 128], F32, tag="acc",
                                        name=f"attnT{qt}_{h}")
                             for h in range(HPC)]
                    for j in range(nk):
                        st = dict(start=(j == 0), stop=(j == nk - 1))
                        for h in range(HPC):
                            nc.tensor.matmul(
                                attnT[h][:],
                                lhsT=v_sb[:, (kt0 + j) * 128:(kt0 + j + 1) * 128],
                                rhs=PnT[:, h * 640 + j * 128:h * 640 + (j + 1) * 128],
                                **st)
                    a_sb = []
                    for h in range(HPC):
                        at = pA.tile([128, 128], MMDT, tag="at", name=f"at{qt}_{h}")
                        nc.vector.tensor_copy(at[:], attnT[h][:])
                        a_sb.append(at[:])
                    ot = pOut.tile([128, D], mybir.dt.bfloat16, tag="ot",
                                   name=f"ot{qt}")
                    for ep in range(2):          # e-chunk pairs, h-outer
                        pr = [psAcc.tile([128, 512], F32, tag="acc",
                                         name=f"pr{qt}_{ep}_{i}") for i in range(2)]
                        for h in range(HPC):
                            for i in range(2):
                                e = ep * 2 + i
                                nc.tensor.matmul(
                                    pr[i][:], lhsT=a_sb[h],
                                    rhs=wo_sb[:, h * D + e * 512:
                                              h * D + (e + 1) * 512],
                                    start=(h == 0), stop=(h == HPC - 1))
                        for i in range(2):
                            e = ep * 2 + i
                            if e < 3:
                                nc.scalar.copy(
                                    ot[:, e * 512:(e + 1) * 512], pr[i][:])
                            else:
                                nc.vector.tensor_copy(
                                    ot[:, e * 512:(e + 1) * 512], pr[i][:])
                    nc.sync.dma_start(out_d[qt * 128:(qt + 1) * 128, :], ot[:])

                for qt in range(QT + 1):
                    if qt < QT:
                        stage_a(qt)
                    if qt >= 1:
                        stage_b(qt - 1)
    nc.compile()
    return nc


# ---------------------------------------------------------------- host side

def _np_mm(a):
    """Convert a host array to the matmul wire dtype."""
    import concourse.mybir as _mb
    return np.ascontiguousarray(a).astype(_mb.dt.np(MMDT))


def _host_prep(x, Wc, bc, Wk, bk, Wv, bv, Wq, bq, Wo, bo, loop_idx):
    f = np.float32
    x = np.asarray(x, f).reshape(T, D)
    Wc, Wk, Wv, Wq, Wo = (np.asarray(a, f) for a in (Wc, Wk, Wv, Wq, Wo))
    bc, bk, bv, bq, bo = (np.asarray(a, f) for a in (bc, bk, bv, bq, bo))
    li = int(np.asarray(loop_idx))

    xT = np.ascontiguousarray(x.T)
    Wck = Wc @ Wk
    bck = bc @ Wk + bk
    Wcv = Wc @ Wv
    bcv = bc @ Wv + bv

    pos = (np.arange(T) + li * T).astype(np.float64)
    inv = 1.0 / (10000.0 ** (np.arange(0, HD, 2).astype(np.float64) / HD))
    ang = pos[:, None] * inv[None, :]                       # [T, 64]
    cos = np.cos(ang).astype(f)
    sin = np.sin(ang).astype(f)
    cos2 = np.ascontiguousarray(np.concatenate([cos, cos], axis=1).T)  # [128, T]
    sin2 = np.ascontiguousarray(np.concatenate([sin, sin], axis=1).T)

    perm = np.concatenate([np.arange(0, HD, 2), np.arange(1, HD, 2)])

    a = np.arange(128)
    tri_lo = np.where(a[None, :] <= a[:, None], 0.0, MASKV).astype(f)  # causal
    tri_hi = np.where(a[None, :] >= a[:, None], 0.0, MASKV).astype(f)
    eye = np.eye(128, dtype=f)
    # block-selector: selrep[32s+b, k] = 1 iff k//BS == b  (replicated 4x on
    # the partition axis so any 32-row slice has base partition 32s)
    blk = np.arange(NB)
    sel1 = (np.arange(T)[None, :] // BS == blk[:, None]).astype(f)   # [32, T]
    selrep = np.tile(sel1, (4, 1))                                    # [128, T]
    J = np.zeros((128, 128), f)
    J[np.arange(64), np.arange(64) + 64] = -1.0
    J[np.arange(64) + 64, np.arange(64)] = 1.0
    jt = np.ascontiguousarray(J.T)

    in_maps = []
    bo_eff = bo.copy()
    for c in range(NCORE):
        h0 = HPC * c
        g = h0 // (NH // NKV)
        Wq_c = Wq[:, h0 * HD:(h0 + HPC) * HD].reshape(D, HPC, HD)[:, :, perm]
        Wq_c = np.ascontiguousarray(Wq_c.reshape(D, HPC * HD))
        bq_c = bq[h0 * HD:(h0 + HPC) * HD].reshape(HPC, HD)[:, perm]
        Wck_c = np.ascontiguousarray(Wck[:, g * HD:(g + 1) * HD][:, perm])
        bck_c = bck[g * HD:(g + 1) * HD][perm]
        Wcv_c = np.ascontiguousarray(Wcv[:, g * HD:(g + 1) * HD])
        bcv_c = bcv[g * HD:(g + 1) * HD]
        Wo_c = np.ascontiguousarray(Wo[h0 * HD:(h0 + HPC) * HD, :])
        # v-bias folded through softmax (rows sum to 1): + bcv @ Wo_head, per head
        for hh in range(HPC):
            bo_eff = bo_eff + bcv_c @ Wo_c[hh * HD:(hh + 1) * HD]
        bias3 = np.stack([bq_c[0], bq_c[1], bck_c], axis=1).astype(f)  # [128, 3]
        in_maps.append({
            "xT": _np_mm(xT), "wq": _np_mm(Wq_c), "wck": _np_mm(Wck_c),
            "wcv": _np_mm(Wcv_c), "wo": _np_mm(Wo_c),
            "cos2": cos2, "sin2": sin2,
            "trihiT": _np_mm(tri_hi.T), "triloT": _np_mm(tri_lo.T),
            "selrep": _np_mm(selrep),
            "eye": eye, "eyeb": _np_mm(eye), "jt": _np_mm(jt), "bias3": bias3,
        })
    return in_maps, bo_eff


def _maybe_install_ntff_hook():
    """This axon image lacks antenv.axon_hooks; synthesize it so
    run_bass_kernel_spmd(trace=True) can capture NTFFs. Best-effort."""
    try:
        import sys
        import types
        import antenv
        if getattr(antenv, "axon_hooks", None) is not None:
            return
        from trn_agent_boot.trn_boot import _ntff_profile_via_ctypes
        hook = _ntff_profile_via_ctypes("/opt/axon/libaxon_pjrt.so")
        mod = types.ModuleType("antenv.axon_hooks")
        mod._hook = hook
        mod.get_axon_ntff_profile_hook = lambda: mod._hook
        mod.set_axon_ntff_profile_hook = lambda h: setattr(mod, "_hook", h)
        sys.modules["antenv.axon_hooks"] = mod
        antenv.axon_hooks = mod
    except Exception as e:  # profiling is optional
        print(f"ntff hook install failed: {e}")


def kernel(**inputs) -> np.ndarray:
    in_maps, bo_eff = _host_prep(**inputs)
    if "nc" not in _cache:
        _cache["nc"] = build_nc()
    trace = bool(int(os.environ.get("KERNEL_TRACE", "0")))
    if trace:
        _maybe_install_ntff_hook()
    res = run_bass_kernel_spmd(
        _cache["nc"], in_maps, core_ids=list(range(NCORE)),
        trace=trace)
    if trace:
        _cache["last_results"] = res
    out = np.zeros((T, D), np.float64)
    for r in res.results:
        out += r["out"].astype(np.float64)
    out = (out + bo_eff.astype(np.float64)).astype(np.float32)
    return out.reshape(B, T, D)


# revision 13
# speedup vs baseline: 1.0329x; 1.0329x over previous
"""Trainium2 Bass kernel for nn_DSA2Attention (MLA-latent sparse sliding-window attention).

Strategy (tensor-parallel over heads, 8 cores, 2 heads/core):
  host:  fold Wc into Wk/Wv (k = x @ (Wc@Wk) etc), permute q/k head-dims so rope
         pairs become [x1(0:64); x2(64:128)], precompute rope cos/sin tables in
         [d', t] layout, transposed triangle masks (bf16), a block-selector
         matrix sel_rep, identity.
  device (per core, SPMD — identical program, different weight slices):
    phase 1: qT[d,t], kT[d,t] (feature-major) and v[t,d] via PE matmuls from
             xT chunks (DMA interleaved per k-tile so the PE starts early);
             rope on DVE; block-mean kbT via segmented reduce.
    bulk:    per (qt, h): block scores bsc = qT.T@kbT; top-16-of-32 via
             max8/match_replace; boost vector -> boost_all; every 4 chunks
             PE-transpose boost_all -> boostT (for the boost matmul below).
    phase 2: per query-tile of 128: S = qT.T@kT over <=5 key tiles in PSUM;
             boost broadcast-add and triangle masks are MATMULS accumulated
             into the same PSUM group (lhsT=boostT slice, rhs=sel_rep slice;
             lhsT=triT, rhs=I) — no DVE/GpSimd touch of S;
             exp (scores bounded; no row-max) PSUM->SBUF bf16 with accumulated
             row sums; normalize on DVE (bf16 4x mode); P.T via xbar DMA
             transpose (SBUF->SBUF, no PE); AV accumulate attnT[d,q];
             out-projection psum -> DMA straight to DRAM.
  host:  sum the 8 partial projections (row-parallel Wo) + bias.

Numerics: matmul operands in bf16 (fp32 PSUM accumulation), softmax chain in
fp32 except P in bf16, output partials bf16 summed in fp64 on host.
"""
import os
import numpy as np

import concourse.bacc as bacc
import concourse.bass as bass
import concourse.mybir as mybir
import concourse.tile as tile
from concourse.bass_utils import run_bass_kernel_spmd

B, T, D = 1, 2048, 2048
NH, NKV, HD = 16, 4, 128
KVC = 512
WIN = 512
BS = 64
NSEL = 16
SCALE = HD ** -0.5
NB = T // BS          # 32
NCORE = 8
HPC = NH // NCORE     # heads per core = 2

KT = T // 128         # 16 k-tiles
NCH = 4               # phase-1 t-chunks
CH = T // NCH         # 512
QT = T // 128         # 16 query tiles
NEG = -1e30
MASKV = -1e30 / SCALE

F32 = mybir.dt.float32
AF = mybir.ActivationFunctionType
OP = mybir.AluOpType

MM_DT = os.environ.get("MM_DT", "bf16")
MMDT = {"bf16": mybir.dt.bfloat16, "f32": F32}[MM_DT]

_cache = {}


def build_nc(trace_label=""):
    nc = bacc.Bacc("TRN2", target_bir_lowering=False, debug=False, num_devices=NCORE)

    xT_d = nc.dram_tensor("xT", [D, T], MMDT, kind="ExternalInput").ap()
    wq_d = nc.dram_tensor("wq", [D, HPC * HD], MMDT, kind="ExternalInput").ap()
    wck_d = nc.dram_tensor("wck", [D, HD], MMDT, kind="ExternalInput").ap()
    wcv_d = nc.dram_tensor("wcv", [D, HD], MMDT, kind="ExternalInput").ap()
    wo_d = nc.dram_tensor("wo", [HPC * HD, D], MMDT, kind="ExternalInput").ap()
    cos2_d = nc.dram_tensor("cos2", [HD, T], F32, kind="ExternalInput").ap()
    sin2_d = nc.dram_tensor("sin2", [HD, T], F32, kind="ExternalInput").ap()
    trihiT_d = nc.dram_tensor("trihiT", [128, 128], MMDT, kind="ExternalInput").ap()
    triloT_d = nc.dram_tensor("triloT", [128, 128], MMDT, kind="ExternalInput").ap()
    selrep_d = nc.dram_tensor("selrep", [128, T], MMDT, kind="ExternalInput").ap()
    eye_d = nc.dram_tensor("eye", [128, 128], F32, kind="ExternalInput").ap()
    eyeb_d = nc.dram_tensor("eyeb", [128, 128], MMDT, kind="ExternalInput").ap()
    jt_d = nc.dram_tensor("jt", [128, 128], MMDT, kind="ExternalInput").ap()
    bias3_d = nc.dram_tensor("bias3", [HD, 3], F32, kind="ExternalInput").ap()
    out_d = nc.dram_tensor("out", [T, D], mybir.dt.bfloat16,
                           kind="ExternalOutput").ap()

    with tile.TileContext(nc) as tc:
        with tc.tile_pool(name="persist", bufs=1) as pp:
            wq_sb = pp.tile([128, KT * HPC * HD], MMDT, tag="wq")
            wck_sb = pp.tile([128, KT * HD], MMDT, tag="wck")
            wcv_sb = pp.tile([128, KT * HD], MMDT, tag="wcv")
            wo_sb = pp.tile([128, HPC * D], MMDT, tag="wo")
            cos2_sb = pp.tile([128, T], F32, tag="cos2")
            sin2_sb = pp.tile([128, T], F32, tag="sin2")
            trihiT = pp.tile([128, 128], MMDT, tag="trihiT")
            triloT = pp.tile([128, 128], MMDT, tag="triloT")
            selrep = pp.tile([128, T], MMDT, tag="selrep")
            eye_sb = pp.tile([128, 128], F32, tag="eye")
            eyeb = pp.tile([128, 128], MMDT, tag="eyeb")
            jt_sb = pp.tile([128, 128], MMDT, tag="jt")
            bias3 = pp.tile([128, 3], F32, tag="bias3")
            qT_sb = pp.tile([128, HPC * T], MMDT, tag="qT")
            kT_sb = pp.tile([128, T], MMDT, tag="kT")
            v_sb = pp.tile([128, KT * HD], MMDT, tag="v")
            kbT = pp.tile([128, NB], MMDT, tag="kbT")
            boost_all = pp.tile([128, QT * HPC * NB], F32, tag="boost_all")
            boostT = pp.tile([128, QT * HPC * NB], MMDT, tag="boostT")

            # ---------- phase 1 DMA: interleave weights and x per k-tile so
            # the first matmuls can start after ~200KB instead of ~3MB.
            xt_sb = pp.tile([128, KT * T], MMDT, tag="xt")
            vT_sb = pp.tile([128, T], MMDT, tag="vT")
            for kt in range(KT):
                nc.sync.dma_start(
                    wq_sb[:, kt * HPC * HD:(kt + 1) * HPC * HD],
                    wq_d[kt * 128:(kt + 1) * 128, :])
                nc.sync.dma_start(
                    wck_sb[:, kt * HD:(kt + 1) * HD],
                    wck_d[kt * 128:(kt + 1) * 128, :])
                nc.sync.dma_start(
                    wcv_sb[:, kt * HD:(kt + 1) * HD],
                    wcv_d[kt * 128:(kt + 1) * 128, :])
                nc.sync.dma_start(
                    xt_sb[:, kt * T:kt * T + CH],
                    xT_d[kt * 128:(kt + 1) * 128, 0:CH])
            nc.sync.dma_start(bias3[:], bias3_d)
            nc.sync.dma_start(cos2_sb[:], cos2_d)
            nc.sync.dma_start(sin2_sb[:], sin2_d)
            nc.sync.dma_start(jt_sb[:], jt_d)
            nc.sync.dma_start(trihiT[:], trihiT_d)
            nc.sync.dma_start(triloT[:], triloT_d)
            nc.sync.dma_start(selrep[:], selrep_d)
            nc.sync.dma_start(eye_sb[:], eye_d)
            nc.sync.dma_start(eyeb[:], eyeb_d)
            # x ch1-3 and wo go on the ACT hwdge queue (idle during phase 1)
            # so the input load is not serialized on the single Sync queue.
            for ch in range(1, NCH):
                for kt in range(KT):
                    nc.scalar.dma_start(
                        xt_sb[:, kt * T + ch * CH:kt * T + (ch + 1) * CH],
                        xT_d[kt * 128:(kt + 1) * 128, ch * CH:(ch + 1) * CH])
            nc.scalar.dma_start(
                wo_sb[:].rearrange("p (h e) -> p h e", h=HPC),
                wo_d.rearrange("(h p) e -> p h e", p=128))

            # ---------- phase 1 compute: qT, kT, v ----------
            with tc.tile_pool(name="rs", bufs=3) as rsp, \
                 tc.tile_pool(name="psA", bufs=8, space="PSUM") as psA:
                def p1_mms(ch):
                    qd = [psA.tile([128, CH], F32, tag="qkT", name=f"qd{ch}_{_h}")
                          for _h in range(HPC)]
                    kTp = psA.tile([128, CH], F32, tag="qkT", name=f"kTp{ch}")
                    vTp = psA.tile([128, CH], F32, tag="qkT", name=f"vTp{ch}")
                    for kt in range(KT):
                        xt = xt_sb[:, kt * T + ch * CH:kt * T + (ch + 1) * CH]
                        st = dict(start=(kt == 0), stop=(kt == KT - 1))
                        for h in range(HPC):
                            nc.tensor.matmul(
                                qd[h][:],
                                lhsT=wq_sb[:, kt * HPC * HD + h * HD:
                                           kt * HPC * HD + (h + 1) * HD],
                                rhs=xt, **st)
                        nc.tensor.matmul(
                            kTp[:], lhsT=wck_sb[:, kt * HD:(kt + 1) * HD],
                            rhs=xt, **st)
                        nc.tensor.matmul(
                            vTp[:], lhsT=wcv_sb[:, kt * HD:(kt + 1) * HD],
                            rhs=xt, **st)
                    return qd, kTp, vTp

                def p1_rope(ch, qd, kTp, vTp):
                    cs = slice(ch * CH, (ch + 1) * CH)
                    # rope + bias: dst = (ps+b)*cos2 + J @ ((ps+b)*sin2)
                    for ti, (ps, dst) in enumerate(
                            [(qd[0], qT_sb[:, 0 * T + ch * CH:0 * T + (ch + 1) * CH]),
                             (qd[1], qT_sb[:, 1 * T + ch * CH:1 * T + (ch + 1) * CH]),
                             (kTp, kT_sb[:, cs])]):
                        U = rsp.tile([128, CH], F32, tag="U", name=f"U{ch}_{ti}")
                        Wt = rsp.tile([128, CH], MMDT, tag="W", name=f"Wt{ch}_{ti}")
                        b = bias3[:, ti:ti + 1]
                        nc.vector.scalar_tensor_tensor(
                            U[:], ps[:], b, cos2_sb[:, cs], op0=OP.add, op1=OP.mult)
                        nc.vector.scalar_tensor_tensor(
                            Wt[:], ps[:], b, sin2_sb[:, cs], op0=OP.add, op1=OP.mult)
                        rp = psA.tile([128, CH], F32, tag="qkT", name=f"rp{ch}_{ti}")
                        nc.tensor.matmul(rp[:], lhsT=jt_sb[:], rhs=Wt[:],
                                         start=True, stop=True)
                        nc.vector.tensor_add(dst, rp[:], U[:])
                    nc.any.tensor_copy(vT_sb[:, cs], vTp[:])

                prev = None
                for ch in range(NCH):
                    cur = p1_mms(ch)
                    if prev is not None:
                        p1_rope(ch - 1, *prev)
                    prev = cur
                p1_rope(NCH - 1, *prev)

                # v[t, d] from vT[d, t] via one xbar transpose (bf16)
                nc.scalar.dma_start_transpose(
                    v_sb[:].rearrange("p (k f) -> p k f", k=KT), vT_sb[:])

                # block means of roped kT: [128, T] -> [128, NB], 1/BS scale
                with nc.allow_low_precision(reason="bf16 block-mean output"):
                    nc.vector.reduce_sum(
                        kbT[:, :, None],
                        kT_sb[:].rearrange("p (b i) -> p b i", b=NB),
                        axis=mybir.AxisListType.X)
                nc.vector.tensor_scalar_mul(kbT[:], kbT[:], 1.0 / BS)

            # ---------- phase 2: attention + projection ----------
            # Bulk top-k (block scores + top-16 boost + boostT transpose) is
            # emitted INTERLEAVED with the phase-2 stages: each engine's
            # instruction stream is strict FIFO, so emitting all 32 top-k
            # chains first would park ~34us of DVE work ahead of every
            # phase-2 DVE op and starve the PE.
            # Stage A(qt): S qk matmuls + boost/mask matmuls in one PSUM
            # group -> exp (PSUM->SBUF bf16, accum row sums) -> DVE normalize
            # (bf16 4x) -> xbar DMA transpose P -> PnT.
            # Stage B(qt): AV -> projection -> DMA out.
            with tc.tile_pool(name="psS", bufs=2, space="PSUM") as psS, \
                 tc.tile_pool(name="psAcc", bufs=2, space="PSUM") as psAcc, \
                 tc.tile_pool(name="psB", bufs=2, space="PSUM") as psB, \
                 tc.tile_pool(name="pTk", bufs=6) as pTk, \
                 tc.tile_pool(name="pPn", bufs=3) as pPn, \
                 tc.tile_pool(name="pPT", bufs=2) as pPT, \
                 tc.tile_pool(name="pA", bufs=4) as pA, \
                 tc.tile_pool(name="pOut", bufs=2) as pOut, \
                 tc.tile_pool(name="pSm", bufs=8) as pSm:
                pnt_tiles = {}

                def bulk(bqt):
                    for h in range(HPC):
                        c = bqt * HPC + h
                        qTh = qT_sb[:, h * T + bqt * 128:h * T + (bqt + 1) * 128]
                        bsc = psB.tile([128, NB], F32, tag="bsc",
                                       name=f"bsc{bqt}_{h}")
                        nc.tensor.matmul(bsc[:], lhsT=qTh, rhs=kbT[:],
                                         start=True, stop=True)
                        z = pTk.tile([128, NB], F32, tag="z", name=f"z{bqt}_{h}")
                        m8 = pTk.tile([128, 8], F32, tag="m8",
                                      name=f"m8_{bqt}_{h}")
                        nc.vector.tensor_copy(z[:], bsc[:])
                        for _ in range(NSEL // 8):
                            nc.vector.max(out=m8[:], in_=z[:])
                            nc.vector.match_replace(
                                out=z[:], in_to_replace=m8[:], in_values=z[:],
                                imm_value=NEG)
                        bo_sl = boost_all[:, c * NB:(c + 1) * NB]
                        nc.vector.scalar_tensor_tensor(
                            bo_sl, z[:], NEG, bsc[:],
                            op0=OP.is_le, op1=OP.mult)

                def btrans(t):
                    btp = psB.tile([128, 128], F32, tag="bsc", name=f"btp{t}")
                    nc.tensor.transpose(
                        btp[:], boost_all[:, t * 128:(t + 1) * 128], eye_sb[:])
                    nc.vector.tensor_copy(
                        boostT[:, t * 128:(t + 1) * 128], btp[:])

                bulk_next = [0]

                def ensure_boost(tq):
                    t = tq // 2
                    while bulk_next[0] <= 2 * t + 1:
                        bq = bulk_next[0]
                        bulk(bq)
                        bulk_next[0] += 1
                        if bulk_next[0] % 2 == 0:
                            btrans(bulk_next[0] // 2 - 1)

                def stage_a(qt):
                    nk = min(qt, 4) + 1
                    kt0 = qt + 1 - nk
                    t = qt // 2
                    olist = [(0, 512), (512, 128)] if nk == 5 else [(0, nk * 128)]
                    S = [psS.tile([128, 640], F32, tag="S", name=f"S{qt}_{h}")
                         for h in range(HPC)]
                    # qk matmuls (one LDW per head)
                    for h in range(HPC):
                        qTh = qT_sb[:, h * T + qt * 128:h * T + (qt + 1) * 128]
                        for o, w in olist:
                            nc.tensor.matmul(
                                S[h][:, o:o + w], lhsT=qTh,
                                rhs=kT_sb[:, kt0 * 128 + o:kt0 * 128 + o + w],
                                start=True, stop=False)
                    # triangle masks as accumulated matmuls (lhsT=triT, rhs=I)
                    if qt >= 4:
                        for h in range(HPC):
                            nc.tensor.matmul(
                                S[h][:, 0:128], lhsT=trihiT[:], rhs=eyeb[:],
                                start=False, stop=False)
                    for h in range(HPC):
                        nc.tensor.matmul(
                            S[h][:, (nk - 1) * 128:nk * 128], lhsT=triloT[:],
                            rhs=eyeb[:], start=False, stop=False)
                    # boost broadcast-add as matmul: lhsT = boostT chunk
                    # [32, 128q], rhs = selector rows at same partitions
                    for h in range(HPC):
                        c = qt * HPC + h
                        s = (c % 4) * 32
                        bT = boostT[s:s + 32, t * 128:(t + 1) * 128]
                        for o, w in olist:
                            nc.tensor.matmul(
                                S[h][:, o:o + w], lhsT=bT,
                                rhs=selrep[s:s + 32,
                                           kt0 * 128 + o:kt0 * 128 + o + w],
                                start=False, stop=True,
                                tile_position=(s, 0))
                    # exp + row sums; normalize on DVE (bf16 4x); transpose
                    Pn = pPn.tile([128, 2 * 640], MMDT, tag="Pn", name=f"Pn{qt}")
                    PnT = pPT.tile([128, 2 * 640], MMDT, tag="PnT", name=f"PnT{qt}")
                    pnt_tiles[qt] = PnT
                    for h in range(HPC):
                        r = pSm.tile([128, 1], F32, tag="r", name=f"r{qt}_{h}")
                        rinv = pSm.tile([128, 1], F32, tag="rinv",
                                        name=f"rinv{qt}_{h}")
                        pn_h = Pn[:, h * 640:h * 640 + nk * 128]
                        with nc.allow_low_precision(reason="bf16 P with f32 sums"):
                            nc.scalar.activation(
                                pn_h, S[h][:, :nk * 128], AF.Exp,
                                scale=float(SCALE), accum_out=r[:])
                        nc.vector.reciprocal(rinv[:], r[:])
                        nc.vector.tensor_scalar_mul(pn_h, pn_h, rinv[:])
                        nc.sync.dma_start_transpose(
                            PnT[:, h * 640:h * 640 + nk * 128].rearrange(
                                "p (k f) -> p k f", k=nk),
                            pn_h)

                def stage_b(qt):
                    nk = min(qt, 4) + 1
                    kt0 = qt + 1 - nk
                    PnT = pnt_tiles.pop(qt)
                    attnT = [psAcc.tile([128, 128], F32, tag="acc",
                                        name=f"attnT{qt}_{h}")
                             for h in range(HPC)]
                    for j in range(nk):
                        st = dict(start=(j == 0), stop=(j == nk - 1))
                        for h in range(HPC):
                            nc.tensor.matmul(
                                attnT[h][:],
                                lhsT=v_sb[:, (kt0 + j) * 128:(kt0 + j + 1) * 128],
                                rhs=PnT[:, h * 640 + j * 128:h * 640 + (j + 1) * 128],
                                **st)
                    a_sb = []
                    for h in range(HPC):
                        at = pA.tile([128, 128], MMDT, tag="at", name=f"at{qt}_{h}")
                        nc.vector.tensor_copy(at[:], attnT[h][:])
                        a_sb.append(at[:])
                    ot = pOut.tile([128, D], mybir.dt.bfloat16, tag="ot",
                                   name=f"ot{qt}")
                    for ep in range(2):          # e-chunk pairs, h-outer
                        pr = [psAcc.tile([128, 512], F32, tag="acc",
                                         name=f"pr{qt}_{ep}_{i}") for i in range(2)]
                        for h in range(HPC):
                            for i in range(2):
                                e = ep * 2 + i
                                nc.tensor.matmul(
                                    pr[i][:], lhsT=a_sb[h],
                                    rhs=wo_sb[:, h * D + e * 512:
                                              h * D + (e + 1) * 512],
                                    start=(h == 0), stop=(h == HPC - 1))
                        for i in range(2):
                            e = ep * 2 + i
                            if e < 3:
                                nc.scalar.copy(
                                    ot[:, e * 512:(e + 1) * 512], pr[i][:])
                            else:
                                nc.vector.tensor_copy(
                                    ot[:, e * 512:(e + 1) * 512], pr[i][:])
                    nc.gpsimd.dma_start(out_d[qt * 128:(qt + 1) * 128, :], ot[:])

                for qt in range(QT + 1):
                    if qt < QT:
                        ensure_boost(min(qt + 1, QT - 1))
                        stage_a(qt)
                    if qt >= 1:
                        stage_b(qt - 1)
    nc.compile()
    return nc


# ---------------------------------------------------------------- host side

def _np_mm(a):
    """Convert a host array to the matmul wire dtype."""
    import concourse.mybir as _mb
    return np.ascontiguousarray(a).astype(_mb.dt.np(MMDT))


def _host_prep(x, Wc, bc, Wk, bk, Wv, bv, Wq, bq, Wo, bo, loop_idx):
    f = np.float32
    x = np.asarray(x, f).reshape(T, D)
    Wc, Wk, Wv, Wq, Wo = (np.asarray(a, f) for a in (Wc, Wk, Wv, Wq, Wo))
    bc, bk, bv, bq, bo = (np.asarray(a, f) for a in (bc, bk, bv, bq, bo))
    li = int(np.asarray(loop_idx))

    xT = np.ascontiguousarray(x.T)
    Wck = Wc @ Wk
    bck = bc @ Wk + bk
    Wcv = Wc @ Wv
    bcv = bc @ Wv + bv

    pos = (np.arange(T) + li * T).astype(np.float64)
    inv = 1.0 / (10000.0 ** (np.arange(0, HD, 2).astype(np.float64) / HD))
    ang = pos[:, None] * inv[None, :]                       # [T, 64]
    cos = np.cos(ang).astype(f)
    sin = np.sin(ang).astype(f)
    cos2 = np.ascontiguousarray(np.concatenate([cos, cos], axis=1).T)  # [128, T]
    sin2 = np.ascontiguousarray(np.concatenate([sin, sin], axis=1).T)

    perm = np.concatenate([np.arange(0, HD, 2), np.arange(1, HD, 2)])

    a = np.arange(128)
    tri_lo = np.where(a[None, :] <= a[:, None], 0.0, MASKV).astype(f)  # causal
    tri_hi = np.where(a[None, :] >= a[:, None], 0.0, MASKV).astype(f)
    eye = np.eye(128, dtype=f)
    # block-selector: selrep[32s+b, k] = 1 iff k//BS == b  (replicated 4x on
    # the partition axis so any 32-row slice has base partition 32s)
    blk = np.arange(NB)
    sel1 = (np.arange(T)[None, :] // BS == blk[:, None]).astype(f)   # [32, T]
    selrep = np.tile(sel1, (4, 1))                                    # [128, T]
    J = np.zeros((128, 128), f)
    J[np.arange(64), np.arange(64) + 64] = -1.0
    J[np.arange(64) + 64, np.arange(64)] = 1.0
    jt = np.ascontiguousarray(J.T)

    in_maps = []
    bo_eff = bo.copy()
    for c in range(NCORE):
        h0 = HPC * c
        g = h0 // (NH // NKV)
        Wq_c = Wq[:, h0 * HD:(h0 + HPC) * HD].reshape(D, HPC, HD)[:, :, perm]
        Wq_c = np.ascontiguousarray(Wq_c.reshape(D, HPC * HD))
        bq_c = bq[h0 * HD:(h0 + HPC) * HD].reshape(HPC, HD)[:, perm]
        Wck_c = np.ascontiguousarray(Wck[:, g * HD:(g + 1) * HD][:, perm])
        bck_c = bck[g * HD:(g + 1) * HD][perm]
        Wcv_c = np.ascontiguousarray(Wcv[:, g * HD:(g + 1) * HD])
        bcv_c = bcv[g * HD:(g + 1) * HD]
        Wo_c = np.ascontiguousarray(Wo[h0 * HD:(h0 + HPC) * HD, :])
        # v-bias folded through softmax (rows sum to 1): + bcv @ Wo_head, per head
        for hh in range(HPC):
            bo_eff = bo_eff + bcv_c @ Wo_c[hh * HD:(hh + 1) * HD]
        bias3 = np.stack([bq_c[0], bq_c[1], bck_c], axis=1).astype(f)  # [128, 3]
        in_maps.append({
            "xT": _np_mm(xT), "wq": _np_mm(Wq_c), "wck": _np_mm(Wck_c),
            "wcv": _np_mm(Wcv_c), "wo": _np_mm(Wo_c),
            "cos2": cos2, "sin2": sin2,
            "trihiT": _np_mm(tri_hi.T), "triloT": _np_mm(tri_lo.T),
            "selrep": _np_mm(selrep),
            "eye": eye, "eyeb": _np_mm(eye), "jt": _np_mm(jt), "bias3": bias3,
        })
    return in_maps, bo_eff


def _maybe_install_ntff_hook():
    """This axon image lacks antenv.axon_hooks; synthesize it so
    run_bass_kernel_spmd(trace=True) can capture NTFFs. Best-effort."""
    try:
        import sys
        import types
        import antenv
        if getattr(antenv, "axon_hooks", None) is not None:
            return
        from trn_agent_boot.trn_boot import _ntff_profile_via_ctypes
        hook = _ntff_profile_via_ctypes("/opt/axon/libaxon_pjrt.so")
        mod = types.ModuleType("antenv.axon_hooks")
        mod._hook = hook
        mod.get_axon_ntff_profile_hook = lambda: mod._hook
        mod.set_axon_ntff_profile_hook = lambda h: setattr(mod, "_hook", h)
        sys.modules["antenv.axon_hooks"] = mod
        antenv.axon_hooks = mod
    except Exception as e:  # profiling is optional
        print(f"ntff hook install failed: {e}")


def kernel(**inputs) -> np.ndarray:
    in_maps, bo_eff = _host_prep(**inputs)
    if "nc" not in _cache:
        _cache["nc"] = build_nc()
    trace = bool(int(os.environ.get("KERNEL_TRACE", "0")))
    if trace:
        _maybe_install_ntff_hook()
    res = run_bass_kernel_spmd(
        _cache["nc"], in_maps, core_ids=list(range(NCORE)),
        trace=trace)
    if trace:
        _cache["last_results"] = res
    out = np.zeros((T, D), np.float64)
    for r in res.results:
        out += r["out"].astype(np.float64)
    out = (out + bo_eff.astype(np.float64)).astype(np.float32)
    return out.reshape(B, T, D)


# revision 34
# speedup vs baseline: 1.0697x; 1.0356x over previous
"""Trainium2 Bass kernel for nn_DSA2Attention (MLA-latent sparse sliding-window attention).

Strategy (tensor-parallel over heads, 8 cores, 2 heads/core):
  host:  fold Wc into Wk/Wv (k = x @ (Wc@Wk) etc), permute q/k head-dims so rope
         pairs become [x1(0:64); x2(64:128)], precompute rope cos/sin tables in
         [d', t] layout, transposed triangle masks (bf16), a block-selector
         matrix sel_rep, identity.
  device (per core, SPMD — identical program, different weight slices):
    phase 1: qT[d,t], kT[d,t] (feature-major) and v[t,d] via PE matmuls from
             xT chunks (DMA interleaved per k-tile so the PE starts early);
             rope on DVE; block-mean kbT via segmented reduce.
    bulk:    per (qt, h): block scores bsc = qT.T@kbT; top-16-of-32 via
             max8/match_replace; boost vector -> boost_all; every 4 chunks
             PE-transpose boost_all -> boostT (for the boost matmul below).
    phase 2: per query-tile of 128: S = qT.T@kT over <=5 key tiles in PSUM;
             boost broadcast-add and triangle masks are MATMULS accumulated
             into the same PSUM group (lhsT=boostT slice, rhs=sel_rep slice;
             lhsT=triT, rhs=I) — no DVE/GpSimd touch of S;
             exp (scores bounded; no row-max) PSUM->SBUF bf16 with accumulated
             row sums; normalize on DVE (bf16 4x mode); P.T via xbar DMA
             transpose (SBUF->SBUF, no PE); AV accumulate attnT[d,q];
             out-projection psum -> DMA straight to DRAM.
  host:  sum the 8 partial projections (row-parallel Wo) + bias.

Numerics: matmul operands in bf16 (fp32 PSUM accumulation), softmax chain in
fp32 except P in bf16, output partials bf16 summed in fp64 on host.
"""
import os
import numpy as np

import concourse.bacc as bacc
import concourse.bass as bass
import concourse.mybir as mybir
import concourse.tile as tile
from concourse.bass_utils import run_bass_kernel_spmd

B, T, D = 1, 2048, 2048
NH, NKV, HD = 16, 4, 128
KVC = 512
WIN = 512
BS = 64
NSEL = 16
SCALE = HD ** -0.5
NB = T // BS          # 32
NCORE = 8
HPC = NH // NCORE     # heads per core = 2

KT = T // 128         # 16 k-tiles
NCH = 4               # phase-1 t-chunks
CH = T // NCH         # 512
QT = T // 128         # 16 query tiles
NEG = -1e30
MASKV = -1e30 / SCALE

F32 = mybir.dt.float32
AF = mybir.ActivationFunctionType
OP = mybir.AluOpType

MM_DT = os.environ.get("MM_DT", "bf16")
MMDT = {"bf16": mybir.dt.bfloat16, "f32": F32}[MM_DT]

# Phase-1 projections run in fp8e4m3 with DoubleRow (2 K-rows/cell): host
# pre-scales W by XSC so 0.02-magnitude weights use fp8's normal range,
# then folds 1/XSC^2 into the exp scale and 1/XSC into Wo.
P1_F8 = os.environ.get("P1_DT", "bf16") == "f8"
F8 = mybir.dt.float8e4
P1DT = F8 if P1_F8 else MMDT
XSC = 64.0 if P1_F8 else 1.0
DR = mybir.MatmulPerfMode.DoubleRow

_cache = {}


def build_nc(trace_label=""):
    nc = bacc.Bacc("TRN2", target_bir_lowering=False, debug=False, num_devices=NCORE)

    # All inputs arrive in DRAM already in SBUF layout (host pre-arranged):
    # one fat contiguous DMA each, multi-KB per-partition lines.
    xT_d = nc.dram_tensor("xT", [128, KT * T], P1DT, kind="ExternalInput").ap()
    wq_d = nc.dram_tensor("wq", [128, KT * HPC * HD], P1DT,
                          kind="ExternalInput").ap()
    wck_d = nc.dram_tensor("wck", [128, KT * HD], P1DT,
                           kind="ExternalInput").ap()
    wcv_d = nc.dram_tensor("wcv", [128, KT * HD], P1DT,
                           kind="ExternalInput").ap()
    wo_d = nc.dram_tensor("wo", [128, HPC * D], MMDT, kind="ExternalInput").ap()
    cos2_d = nc.dram_tensor("cos2", [HD, T], F32, kind="ExternalInput").ap()
    sin2_d = nc.dram_tensor("sin2", [HD, T], F32, kind="ExternalInput").ap()
    trihiT_d = nc.dram_tensor("trihiT", [128, 128], MMDT, kind="ExternalInput").ap()
    triloT_d = nc.dram_tensor("triloT", [128, 128], MMDT, kind="ExternalInput").ap()
    selrep_d = nc.dram_tensor("selrep", [128, T], MMDT, kind="ExternalInput").ap()
    eye_d = nc.dram_tensor("eye", [128, 128], F32, kind="ExternalInput").ap()
    eyeb_d = nc.dram_tensor("eyeb", [128, 128], MMDT, kind="ExternalInput").ap()
    jt_d = nc.dram_tensor("jt", [128, 128], MMDT, kind="ExternalInput").ap()
    bias3_d = nc.dram_tensor("bias3", [HD, 3], F32, kind="ExternalInput").ap()
    out_d = nc.dram_tensor("out", [T, D], mybir.dt.bfloat16,
                           kind="ExternalOutput").ap()

    with tile.TileContext(nc) as tc:
        with tc.tile_pool(name="persist", bufs=1) as pp:
            wq_sb = pp.tile([128, KT * HPC * HD], P1DT, tag="wq")
            wck_sb = pp.tile([128, KT * HD], P1DT, tag="wck")
            wcv_sb = pp.tile([128, KT * HD], P1DT, tag="wcv")
            wo_sb = pp.tile([128, HPC * D], MMDT, tag="wo")
            cos2_sb = pp.tile([128, T], F32, tag="cos2")
            sin2_sb = pp.tile([128, T], F32, tag="sin2")
            trihiT = pp.tile([128, 128], MMDT, tag="trihiT")
            triloT = pp.tile([128, 128], MMDT, tag="triloT")
            selrep = pp.tile([128, T], MMDT, tag="selrep")
            eye_sb = pp.tile([128, 128], F32, tag="eye")
            eyeb = pp.tile([128, 128], MMDT, tag="eyeb")
            jt_sb = pp.tile([128, 128], MMDT, tag="jt")
            bias3 = pp.tile([128, 3], F32, tag="bias3")
            qT_sb = pp.tile([128, HPC * T], MMDT, tag="qT")
            kT_sb = pp.tile([128, T], MMDT, tag="kT")
            v_sb = pp.tile([128, KT * HD], MMDT, tag="v")
            kbT = pp.tile([128, NB], MMDT, tag="kbT")
            boost_all = pp.tile([128, QT * HPC * NB], F32, tag="boost_all")
            boostT = pp.tile([128, QT * HPC * NB], MMDT, tag="boostT")

            # ---------- phase 1 DMA: everything is a fat contiguous copy;
            # weights+small consts first, then x per k-tile alternating
            # across the two hwdge queues so early tiles land early.
            xt_sb = pp.tile([128, KT * T], P1DT, tag="xt")
            vT_sb = pp.tile([128, T], MMDT, tag="vT")
            nc.sync.dma_start(bias3[:], bias3_d)
            nc.scalar.dma_start(cos2_sb[:], cos2_d)
            nc.scalar.dma_start(sin2_sb[:], sin2_d)
            nc.sync.dma_start(wq_sb[:], wq_d)
            nc.sync.dma_start(wck_sb[:], wck_d)
            nc.sync.dma_start(wcv_sb[:], wcv_d)
            nc.sync.dma_start(jt_sb[:], jt_d)
            nc.sync.dma_start(trihiT[:], trihiT_d)
            nc.sync.dma_start(triloT[:], triloT_d)
            nc.sync.dma_start(selrep[:], selrep_d)
            nc.sync.dma_start(eye_sb[:], eye_d)
            nc.sync.dma_start(eyeb[:], eyeb_d)
            for kt in range(KT):
                eng = nc.sync if kt % 2 == 0 else nc.scalar
                eng.dma_start(xt_sb[:, kt * T:(kt + 1) * T],
                              xT_d[:, kt * T:(kt + 1) * T])
            nc.scalar.dma_start(wo_sb[:], wo_d)

            # ---------- phase 1 compute: qT, kT, v ----------
            with tc.tile_pool(name="rs", bufs=3) as rsp, \
                 tc.tile_pool(name="psA", bufs=8, space="PSUM") as psA:
                xt_k = xt_sb[:].rearrange("p (k t) -> p k t", k=KT)
                wq_k = wq_sb[:].rearrange("p (k m) -> p k m", k=KT)
                wck_k = wck_sb[:].rearrange("p (k m) -> p k m", k=KT)
                wcv_k = wcv_sb[:].rearrange("p (k m) -> p k m", k=KT)

                def p1_mms(ch):
                    qd = [psA.tile([128, CH], F32, tag="qkT", name=f"qd{ch}_{_h}")
                          for _h in range(HPC)]
                    kTp = psA.tile([128, CH], F32, tag="qkT", name=f"kTp{ch}")
                    vTp = psA.tile([128, CH], F32, tag="qkT", name=f"vTp{ch}")
                    if P1_F8:
                        # DoubleRow: contract 2 k-tiles per matmul; both APs
                        # are [128, 2, n] with the pair on the middle axis.
                        for k2 in range(KT // 2):
                            xt = xt_k[:, 2 * k2:2 * k2 + 2,
                                      ch * CH:(ch + 1) * CH]
                            st = dict(start=(k2 == 0), stop=(k2 == KT // 2 - 1),
                                      perf_mode=DR)
                            for h in range(HPC):
                                nc.tensor.matmul(
                                    qd[h][:],
                                    lhsT=wq_k[:, 2 * k2:2 * k2 + 2,
                                              h * HD:(h + 1) * HD],
                                    rhs=xt, **st)
                            nc.tensor.matmul(
                                kTp[:], lhsT=wck_k[:, 2 * k2:2 * k2 + 2, :],
                                rhs=xt, **st)
                            nc.tensor.matmul(
                                vTp[:], lhsT=wcv_k[:, 2 * k2:2 * k2 + 2, :],
                                rhs=xt, **st)
                    else:
                        for kt in range(KT):
                            xt = xt_sb[:, kt * T + ch * CH:kt * T + (ch + 1) * CH]
                            st = dict(start=(kt == 0), stop=(kt == KT - 1))
                            for h in range(HPC):
                                nc.tensor.matmul(
                                    qd[h][:],
                                    lhsT=wq_sb[:, kt * HPC * HD + h * HD:
                                               kt * HPC * HD + (h + 1) * HD],
                                    rhs=xt, **st)
                            nc.tensor.matmul(
                                kTp[:], lhsT=wck_sb[:, kt * HD:(kt + 1) * HD],
                                rhs=xt, **st)
                            nc.tensor.matmul(
                                vTp[:], lhsT=wcv_sb[:, kt * HD:(kt + 1) * HD],
                                rhs=xt, **st)
                    return qd, kTp, vTp

                def p1_rope(ch, qd, kTp, vTp):
                    cs = slice(ch * CH, (ch + 1) * CH)
                    # rope + bias: dst = (ps+b)*cos2 + J @ ((ps+b)*sin2)
                    # k first: kT (and its block means) gate the bulk top-k.
                    for ti, ps, dst in (
                            (2, kTp, kT_sb[:, cs]),
                            (0, qd[0], qT_sb[:, 0 * T + ch * CH:0 * T + (ch + 1) * CH]),
                            (1, qd[1], qT_sb[:, 1 * T + ch * CH:1 * T + (ch + 1) * CH])):
                        U = rsp.tile([128, CH], F32, tag="U", name=f"U{ch}_{ti}")
                        Wt = rsp.tile([128, CH], MMDT, tag="W", name=f"Wt{ch}_{ti}")
                        b = bias3[:, ti:ti + 1]
                        nc.vector.scalar_tensor_tensor(
                            U[:], ps[:], b, cos2_sb[:, cs], op0=OP.add, op1=OP.mult)
                        nc.vector.scalar_tensor_tensor(
                            Wt[:], ps[:], b, sin2_sb[:, cs], op0=OP.add, op1=OP.mult)
                        rp = psA.tile([128, CH], F32, tag="qkT", name=f"rp{ch}_{ti}")
                        nc.tensor.matmul(rp[:], lhsT=jt_sb[:], rhs=Wt[:],
                                         start=True, stop=True)
                        nc.vector.tensor_add(dst, rp[:], U[:])
                        if ti == 2:
                            # per-chunk block means of roped kT (1/BS applied
                            # once at the end over the tiny [128, NB] tile)
                            with nc.allow_low_precision(reason="bf16 means"):
                                nc.vector.reduce_sum(
                                    kbT[:, ch * (NB // NCH):(ch + 1) * (NB // NCH),
                                        None],
                                    kT_sb[:, cs].rearrange(
                                        "p (b i) -> p b i", b=NB // NCH),
                                    axis=mybir.AxisListType.X)
                    nc.any.tensor_copy(vT_sb[:, cs], vTp[:])

                prev = None
                for ch in range(NCH):
                    cur = p1_mms(ch)
                    if prev is not None:
                        p1_rope(ch - 1, *prev)
                    prev = cur
                p1_rope(NCH - 1, *prev)

                # v[t, d] from vT[d, t] via one xbar transpose (bf16)
                nc.scalar.dma_start_transpose(
                    v_sb[:].rearrange("p (k f) -> p k f", k=KT), vT_sb[:])

                nc.vector.tensor_scalar_mul(kbT[:], kbT[:], 1.0 / BS)

            # ---------- phase 2: attention + projection ----------
            # Bulk top-k (block scores + top-16 boost + boostT transpose) is
            # emitted INTERLEAVED with the phase-2 stages: each engine's
            # instruction stream is strict FIFO, so emitting all 32 top-k
            # chains first would park ~34us of DVE work ahead of every
            # phase-2 DVE op and starve the PE.
            # Stage A(qt): S qk matmuls + boost/mask matmuls in one PSUM
            # group -> exp (PSUM->SBUF bf16, accum row sums) -> DVE normalize
            # (bf16 4x) -> xbar DMA transpose P -> PnT.
            # Stage B(qt): AV -> projection -> DMA out.
            with tc.tile_pool(name="psS", bufs=3, space="PSUM") as psS, \
                 tc.tile_pool(name="psAcc", bufs=2, space="PSUM") as psAcc, \
                 tc.tile_pool(name="pTk", bufs=6) as pTk, \
                 tc.tile_pool(name="pPn", bufs=3) as pPn, \
                 tc.tile_pool(name="pPT", bufs=3) as pPT, \
                 tc.tile_pool(name="pA", bufs=4) as pA, \
                 tc.tile_pool(name="pOut", bufs=2) as pOut, \
                 tc.tile_pool(name="pSm", bufs=8) as pSm:
                pnt_tiles = {}

                def bulk(bqt):
                    for h in range(HPC):
                        c = bqt * HPC + h
                        qTh = qT_sb[:, h * T + bqt * 128:h * T + (bqt + 1) * 128]
                        bsc = psAcc.tile([128, NB], F32, tag="acc",
                                         name=f"bsc{bqt}_{h}")
                        nc.tensor.matmul(bsc[:], lhsT=qTh, rhs=kbT[:],
                                         start=True, stop=True)
                        z = pTk.tile([128, NB], F32, tag="z", name=f"z{bqt}_{h}")
                        m8 = pTk.tile([128, 8], F32, tag="m8",
                                      name=f"m8_{bqt}_{h}")
                        nc.vector.tensor_copy(z[:], bsc[:])
                        for _ in range(NSEL // 8):
                            nc.vector.max(out=m8[:], in_=z[:])
                            nc.vector.match_replace(
                                out=z[:], in_to_replace=m8[:], in_values=z[:],
                                imm_value=NEG)
                        bo_sl = boost_all[:, c * NB:(c + 1) * NB]
                        nc.vector.scalar_tensor_tensor(
                            bo_sl, z[:], NEG, bsc[:],
                            op0=OP.is_le, op1=OP.mult)

                def btrans(t):
                    btp = psAcc.tile([128, 128], F32, tag="acc", name=f"btp{t}")
                    nc.tensor.transpose(
                        btp[:], boost_all[:, t * 128:(t + 1) * 128], eye_sb[:])
                    nc.vector.tensor_copy(
                        boostT[:, t * 128:(t + 1) * 128], btp[:])

                bulk_next = [0]

                def ensure_boost(tq):
                    t = tq // 2
                    while bulk_next[0] <= 2 * t + 1:
                        bq = bulk_next[0]
                        bulk(bq)
                        bulk_next[0] += 1
                        if bulk_next[0] % 2 == 0:
                            btrans(bulk_next[0] // 2 - 1)

                def stage_a(qt):
                    nk = min(qt, 4) + 1
                    kt0 = qt + 1 - nk
                    t = qt // 2
                    olist = [(0, 512), (512, 128)] if nk == 5 else [(0, nk * 128)]
                    S = [psS.tile([128, 640], F32, tag="S", name=f"S{qt}_{h}")
                         for h in range(HPC)]
                    # qk matmuls (one LDW per head)
                    for h in range(HPC):
                        qTh = qT_sb[:, h * T + qt * 128:h * T + (qt + 1) * 128]
                        for o, w in olist:
                            nc.tensor.matmul(
                                S[h][:, o:o + w], lhsT=qTh,
                                rhs=kT_sb[:, kt0 * 128 + o:kt0 * 128 + o + w],
                                start=True, stop=False)
                    # triangle masks as accumulated matmuls (lhsT=triT, rhs=I)
                    if qt >= 4:
                        for h in range(HPC):
                            nc.tensor.matmul(
                                S[h][:, 0:128], lhsT=trihiT[:], rhs=eyeb[:],
                                start=False, stop=False)
                    for h in range(HPC):
                        nc.tensor.matmul(
                            S[h][:, (nk - 1) * 128:nk * 128], lhsT=triloT[:],
                            rhs=eyeb[:], start=False, stop=False)
                    # boost broadcast-add as matmul: lhsT = boostT chunk
                    # [32, 128q], rhs = selector rows at same partitions
                    for h in range(HPC):
                        c = qt * HPC + h
                        s = (c % 4) * 32
                        bT = boostT[s:s + 32, t * 128:(t + 1) * 128]
                        for o, w in olist:
                            nc.tensor.matmul(
                                S[h][:, o:o + w], lhsT=bT,
                                rhs=selrep[s:s + 32,
                                           kt0 * 128 + o:kt0 * 128 + o + w],
                                start=False, stop=True,
                                tile_position=(s, 0))
                    # exp + row sums; normalize on DVE (bf16 4x); transpose
                    Pn = pPn.tile([128, 2 * 640], MMDT, tag="Pn", name=f"Pn{qt}")
                    PnT = pPT.tile([128, 2 * 640], MMDT, tag="PnT", name=f"PnT{qt}")
                    pnt_tiles[qt] = PnT
                    for h in range(HPC):
                        r = pSm.tile([128, 1], F32, tag="r", name=f"r{qt}_{h}")
                        rinv = pSm.tile([128, 1], F32, tag="rinv",
                                        name=f"rinv{qt}_{h}")
                        pn_h = Pn[:, h * 640:h * 640 + nk * 128]
                        with nc.allow_low_precision(reason="bf16 P with f32 sums"):
                            nc.scalar.activation(
                                pn_h, S[h][:, :nk * 128], AF.Exp,
                                scale=float(SCALE / (XSC * XSC)),
                                accum_out=r[:])
                        nc.vector.reciprocal(rinv[:], r[:])
                        nc.vector.tensor_scalar_mul(pn_h, pn_h, rinv[:])
                        nc.sync.dma_start_transpose(
                            PnT[:, h * 640:h * 640 + nk * 128].rearrange(
                                "p (k f) -> p k f", k=nk),
                            pn_h)

                def stage_b(qt):
                    nk = min(qt, 4) + 1
                    kt0 = qt + 1 - nk
                    PnT = pnt_tiles.pop(qt)
                    attnT = [psAcc.tile([128, 128], F32, tag="acc",
                                        name=f"attnT{qt}_{h}")
                             for h in range(HPC)]
                    for j in range(nk):
                        st = dict(start=(j == 0), stop=(j == nk - 1))
                        for h in range(HPC):
                            nc.tensor.matmul(
                                attnT[h][:],
                                lhsT=v_sb[:, (kt0 + j) * 128:(kt0 + j + 1) * 128],
                                rhs=PnT[:, h * 640 + j * 128:h * 640 + (j + 1) * 128],
                                **st)
                    a_sb = []
                    for h in range(HPC):
                        at = pA.tile([128, 128], MMDT, tag="at", name=f"at{qt}_{h}")
                        nc.vector.tensor_copy(at[:], attnT[h][:])
                        a_sb.append(at[:])
                    ot = pOut.tile([128, D], mybir.dt.bfloat16, tag="ot",
                                   name=f"ot{qt}")
                    for ep in range(2):          # e-chunk pairs, h-outer
                        pr = [psAcc.tile([128, 512], F32, tag="acc",
                                         name=f"pr{qt}_{ep}_{i}") for i in range(2)]
                        for h in range(HPC):
                            for i in range(2):
                                e = ep * 2 + i
                                nc.tensor.matmul(
                                    pr[i][:], lhsT=a_sb[h],
                                    rhs=wo_sb[:, h * D + e * 512:
                                              h * D + (e + 1) * 512],
                                    start=(h == 0), stop=(h == HPC - 1))
                        for i in range(2):
                            e = ep * 2 + i
                            if e < 3:
                                nc.scalar.copy(
                                    ot[:, e * 512:(e + 1) * 512], pr[i][:])
                            else:
                                nc.vector.tensor_copy(
                                    ot[:, e * 512:(e + 1) * 512], pr[i][:])
                    eng = nc.sync if qt % 2 == 0 else nc.gpsimd
                    eng.dma_start(out_d[qt * 128:(qt + 1) * 128, :], ot[:])

                for qt in range(QT + 1):
                    if qt < QT:
                        ensure_boost(min(qt + 1, QT - 1))
                        stage_a(qt)
                    if qt >= 1:
                        stage_b(qt - 1)
    nc.compile()
    return nc


# ---------------------------------------------------------------- host side

def _np_mm(a):
    """Convert a host array to the matmul wire dtype."""
    import concourse.mybir as _mb
    return np.ascontiguousarray(a).astype(_mb.dt.np(MMDT))


def _np_p1(a):
    """Convert a phase-1 operand to the projection wire dtype (fp8/bf16)."""
    import concourse.mybir as _mb
    return np.ascontiguousarray(np.clip(a, -440, 440)).astype(_mb.dt.np(P1DT))


def _sb_layout(a):
    """[KT*128, W] row-major -> [128, KT*W] SBUF layout (partition-major)."""
    w = a.shape[1]
    return a.reshape(KT, 128, w).transpose(1, 0, 2).reshape(128, KT * w)


def _host_prep(x, Wc, bc, Wk, bk, Wv, bv, Wq, bq, Wo, bo, loop_idx):
    f = np.float32
    x = np.asarray(x, f).reshape(T, D)
    Wc, Wk, Wv, Wq, Wo = (np.asarray(a, f) for a in (Wc, Wk, Wv, Wq, Wo))
    bc, bk, bv, bq, bo = (np.asarray(a, f) for a in (bc, bk, bv, bq, bo))
    li = int(np.asarray(loop_idx))

    xT = np.ascontiguousarray(x.T)
    Wck = Wc @ Wk
    bck = bc @ Wk + bk
    Wcv = Wc @ Wv
    bcv = bc @ Wv + bv

    pos = (np.arange(T) + li * T).astype(np.float64)
    inv = 1.0 / (10000.0 ** (np.arange(0, HD, 2).astype(np.float64) / HD))
    ang = pos[:, None] * inv[None, :]                       # [T, 64]
    cos = np.cos(ang).astype(f)
    sin = np.sin(ang).astype(f)
    cos2 = np.ascontiguousarray(np.concatenate([cos, cos], axis=1).T)  # [128, T]
    sin2 = np.ascontiguousarray(np.concatenate([sin, sin], axis=1).T)

    perm = np.concatenate([np.arange(0, HD, 2), np.arange(1, HD, 2)])

    a = np.arange(128)
    tri_lo = np.where(a[None, :] <= a[:, None], 0.0, MASKV).astype(f)  # causal
    tri_hi = np.where(a[None, :] >= a[:, None], 0.0, MASKV).astype(f)
    eye = np.eye(128, dtype=f)
    # block-selector: selrep[32s+b, k] = 1 iff k//BS == b  (replicated 4x on
    # the partition axis so any 32-row slice has base partition 32s)
    blk = np.arange(NB)
    sel1 = (np.arange(T)[None, :] // BS == blk[:, None]).astype(f)   # [32, T]
    selrep = np.tile(sel1, (4, 1))                                    # [128, T]
    J = np.zeros((128, 128), f)
    J[np.arange(64), np.arange(64) + 64] = -1.0
    J[np.arange(64) + 64, np.arange(64)] = 1.0
    jt = np.ascontiguousarray(J.T)

    in_maps = []
    bo_eff = bo.copy()
    xT_wire = _np_p1(_sb_layout(xT))          # identical for every core
    for c in range(NCORE):
        h0 = HPC * c
        g = h0 // (NH // NKV)
        Wq_c = Wq[:, h0 * HD:(h0 + HPC) * HD].reshape(D, HPC, HD)[:, :, perm]
        Wq_c = np.ascontiguousarray(Wq_c.reshape(D, HPC * HD))
        bq_c = bq[h0 * HD:(h0 + HPC) * HD].reshape(HPC, HD)[:, perm]
        Wck_c = np.ascontiguousarray(Wck[:, g * HD:(g + 1) * HD][:, perm])
        bck_c = bck[g * HD:(g + 1) * HD][perm]
        Wcv_c = np.ascontiguousarray(Wcv[:, g * HD:(g + 1) * HD])
        bcv_c = bcv[g * HD:(g + 1) * HD]
        Wo_c = np.ascontiguousarray(Wo[h0 * HD:(h0 + HPC) * HD, :])
        # v-bias folded through softmax (rows sum to 1): + bcv @ Wo_head, per head
        for hh in range(HPC):
            bo_eff = bo_eff + bcv_c @ Wo_c[hh * HD:(hh + 1) * HD]
        # device values are XSC-scaled (fp8 range); exp scale and Wo undo it
        bias3 = np.stack([bq_c[0], bq_c[1], bck_c], axis=1).astype(f) * XSC
        Wo_r = Wo_c.reshape(HPC, 128, D).transpose(1, 0, 2).reshape(128, HPC * D)
        in_maps.append({
            "xT": xT_wire, "wq": _np_p1(_sb_layout(Wq_c) * XSC),
            "wck": _np_p1(_sb_layout(Wck_c) * XSC),
            "wcv": _np_p1(_sb_layout(Wcv_c) * XSC),
            "wo": _np_mm(Wo_r / XSC),
            "cos2": cos2, "sin2": sin2,
            "trihiT": _np_mm(tri_hi.T), "triloT": _np_mm(tri_lo.T),
            "selrep": _np_mm(selrep),
            "eye": eye, "eyeb": _np_mm(eye), "jt": _np_mm(jt), "bias3": bias3,
        })
    return in_maps, bo_eff


def _maybe_install_ntff_hook():
    """This axon image lacks antenv.axon_hooks; synthesize it so
    run_bass_kernel_spmd(trace=True) can capture NTFFs. Best-effort."""
    try:
        import sys
        import types
        import antenv
        if getattr(antenv, "axon_hooks", None) is not None:
            return
        from trn_agent_boot.trn_boot import _ntff_profile_via_ctypes
        hook = _ntff_profile_via_ctypes("/opt/axon/libaxon_pjrt.so")
        mod = types.ModuleType("antenv.axon_hooks")
        mod._hook = hook
        mod.get_axon_ntff_profile_hook = lambda: mod._hook
        mod.set_axon_ntff_profile_hook = lambda h: setattr(mod, "_hook", h)
        sys.modules["antenv.axon_hooks"] = mod
        antenv.axon_hooks = mod
    except Exception as e:  # profiling is optional
        print(f"ntff hook install failed: {e}")


def kernel(**inputs) -> np.ndarray:
    in_maps, bo_eff = _host_prep(**inputs)
    if "nc" not in _cache:
        _cache["nc"] = build_nc()
    trace = bool(int(os.environ.get("KERNEL_TRACE", "0")))
    if trace:
        _maybe_install_ntff_hook()
    res = run_bass_kernel_spmd(
        _cache["nc"], in_maps, core_ids=list(range(NCORE)),
        trace=trace)
    if trace:
        _cache["last_results"] = res
    out = np.zeros((T, D), np.float64)
    for r in res.results:
        out += r["out"].astype(np.float64)
    out = (out + bo_eff.astype(np.float64)).astype(np.float32)
    return out.reshape(B, T, D)
